# revision 3
# baseline (speedup 1.0000x reference)
import numpy as np

# GCN 3-layer kernel for trn2, 8 cores.
# N=100000 nodes, F=128 feats, E=1.6M edges (+N self loops).
# Sharding: destination-node rows split 8 ways (12500/core).
# Host: add self-loops, compute sym-norm, sort edges by dst, fold norm into
#   per-chunk one-hot scatter blocks S, pre-gather source rows per layer.
# Device per core, per layer, per 128-dst-node window w (98 windows):
#   psum1[k,n] += xg_chunk[e,k]^T @ S_chunk[e,n]   (K chunks: scatter-add)
#   out[n,f]    = aggT[k,n]^T    @ W[k,f]          (dense transform)
# bias + relu applied on host between launches (elementwise, negligible).

N = 100000
F = 128
NC = 8
NPC = N // NC          # 12500 nodes per core
WIN = 128              # dst nodes per window
NW = (NPC + WIN - 1) // WIN   # 98 windows (12544 padded rows)
NPAD = NW * WIN

_prep = None
_prog = None
EXEC_NS = []


def _preprocess(edge_index):
    src = edge_index[0].astype(np.int64)
    dst = edge_index[1].astype(np.int64)
    loop = np.arange(N, dtype=np.int64)
    src = np.concatenate([src, loop])
    dst = np.concatenate([dst, loop])
    deg = np.bincount(dst, minlength=N).astype(np.float32)
    dinv = np.where(deg > 0, 1.0 / np.sqrt(deg), 0.0).astype(np.float32)
    norm = dinv[src] * dinv[dst]

    order = np.argsort(dst, kind="stable")
    src_s = src[order].astype(np.int32)
    dst_s = dst[order].astype(np.int32)
    norm_s = norm[order]

    core_bounds = np.searchsorted(dst_s, np.arange(NC + 1) * NPC)

    # per-core, per-window edge counts -> global K (chunks of 128 edges/window)
    K = 1
    win_info = []
    for c in range(NC):
        a, b = core_bounds[c], core_bounds[c + 1]
        dl = dst_s[a:b] - c * NPC
        w = dl // WIN
        cnt = np.bincount(w, minlength=NW)
        wb = np.concatenate([[0], np.cumsum(cnt)])
        K = max(K, int(np.max((cnt + 127) // 128)))
        win_info.append((a, dl, wb))
    R = NW * K * 128   # padded edge rows per core

    gidx = np.zeros((NC, R), dtype=np.int64)
    S = np.zeros((NC, R, WIN), dtype=np.float32)
    for c in range(NC):
        a, dl, wb = win_info[c]
        for w in range(NW):
            s0, s1 = wb[w], wb[w + 1]
            n = s1 - s0
            if n == 0:
                continue
            base = w * K * 128
            pos = base + np.arange(n)
            gidx[c, pos] = src_s[a + s0:a + s1]
            S[c, pos, dl[s0:s1] - w * WIN] = norm_s[a + s0:a + s1]
    return K, R, gidx, S


def _build_program(K, R):
    import concourse.bass as bass
    import concourse.mybir as mybir

    nc = bass.Bass()
    f32 = mybir.dt.float32
    xg_d = nc.dram_tensor("xg", [R, F], f32, kind="ExternalInput")
    S_d = nc.dram_tensor("S", [R, WIN], f32, kind="ExternalInput")
    W_d = nc.dram_tensor("W", [F, F], f32, kind="ExternalInput")
    out_d = nc.dram_tensor("out", [NPAD, F], f32, kind="ExternalOutput")

    NBUF = 3

    def ap(t, off, p, f):
        return bass.AP(t, off, [[f, p], [1, f]])

    with (
        nc.semaphore("dma_in") as dma_in,
        nc.semaphore("wsem") as wsem,
        nc.semaphore("mm1") as mm1,
        nc.semaphore("cp1") as cp1,
        nc.semaphore("mm2") as mm2,
        nc.semaphore("cp2") as cp2,
        nc.semaphore("dmo") as dmo,
        nc.sbuf_tensor("xgb", [128, F * NBUF], f32) as xgb,
        nc.sbuf_tensor("Sb", [128, WIN * NBUF], f32) as Sb,
        nc.sbuf_tensor("Wb", [128, F], f32) as Wb,
        nc.sbuf_tensor("aggT", [128, WIN * 2], f32) as aggT,
        nc.sbuf_tensor("ob", [128, F * 2], f32) as ob,
        nc.sbuf_tensor("zero", [128, F], f32) as zero,
        nc.psum_tensor("ps1", [128, WIN], f32) as ps1,
        nc.psum_tensor("ps2", [128, F], f32) as ps2,
    ):
        with nc.Block() as block:

            @block.sync
            def _(sync):
                sync.dma_start(out=ap(Wb, 0, 128, F),
                               in_=ap(W_d, 0, 128, F)).then_inc(wsem, 16)
                for w in range(NW):
                    for c in range(K):
                        i = w * K + c
                        sl = i % NBUF
                        if i >= NBUF:
                            sync.wait_ge(mm1, i - NBUF + 1)
                        sync.dma_start(
                            out=bass.AP(xgb, sl * F, [[F * NBUF, 128], [1, F]]),
                            in_=ap(xg_d, i * 128 * F, 128, F),
                        ).then_inc(dma_in, 16)
                        sync.dma_start(
                            out=bass.AP(Sb, sl * WIN, [[WIN * NBUF, 128], [1, WIN]]),
                            in_=ap(S_d, i * 128 * WIN, 128, WIN),
                        ).then_inc(dma_in, 16)

            @block.gpsimd
            def _(gpsimd):
                gpsimd.memset(ap(zero, 0, 128, F), 0)
                for w in range(NW):
                    gpsimd.wait_ge(cp2, w + 1)
                    gpsimd.dma_start(
                        out=ap(out_d, w * 128 * F, 128, F),
                        in_=bass.AP(ob, (w % 2) * F, [[F * 2, 128], [1, F]]),
                    ).then_inc(dmo, 16)

            @block.tensor
            def _(tensor):
                tensor.wait_ge(wsem, 16)
                for w in range(NW):
                    for c in range(K):
                        i = w * K + c
                        sl = i % NBUF
                        tensor.wait_ge(dma_in, 32 * (i + 1))
                        tensor.matmul(
                            ap(ps1, 0, 128, WIN),
                            bass.AP(xgb, sl * F, [[F * NBUF, 128], [1, F]]),
                            bass.AP(Sb, sl * WIN, [[WIN * NBUF, 128], [1, WIN]]),
                            start=(c == 0), stop=(c == K - 1),
                        ).then_inc(mm1)
                    tensor.wait_ge(cp1, w + 1)
                    if w >= 1:
                        tensor.wait_ge(cp2, w)
                    tensor.matmul(
                        ap(ps2, 0, 128, F),
                        bass.AP(aggT, (w % 2) * WIN, [[WIN * 2, 128], [1, WIN]]),
                        ap(Wb, 0, 128, F),
                        start=True, stop=True,
                    ).then_inc(mm2)

            @block.vector
            def _(vector):
                for w in range(NW):
                    vector.wait_ge(mm1, (w + 1) * K)
                    if w >= 2:
                        vector.wait_ge(mm2, w - 1)
                    vector.tensor_add(
                        bass.AP(aggT, (w % 2) * WIN, [[WIN * 2, 128], [1, WIN]]),
                        bass.AP(zero, 0, [[F, 128], [1, WIN]]),
                        ap(ps1, 0, 128, WIN),
                    ).then_inc(cp1)
                    vector.wait_ge(mm2, w + 1)
                    if w >= 2:
                        vector.wait_ge(dmo, 16 * (w - 1))
                    vector.tensor_add(
                        bass.AP(ob, (w % 2) * F, [[F * 2, 128], [1, F]]),
                        ap(zero, 0, 128, F),
                        ap(ps2, 0, 128, F),
                    ).then_inc(cp2)

    return nc


def _run_layer(prog, act, gidx, S, W):
    from concourse.bass_utils import run_bass_kernel_spmd
    in_maps = []
    for c in range(NC):
        in_maps.append({
            "xg": np.ascontiguousarray(act[gidx[c]]),
            "S": np.ascontiguousarray(S[c]),
            "W": np.ascontiguousarray(W),
        })
    res = run_bass_kernel_spmd(prog, in_maps, list(range(NC)))
    if getattr(res, "exec_time_ns", None):
        EXEC_NS.append(res.exec_time_ns)
    outs = []
    for c in range(NC):
        r = res.results[c]
        if isinstance(r, dict):
            r = r["out"]
        elif isinstance(r, (list, tuple)):
            r = r[0]
        outs.append(np.asarray(r)[:NPC])
    return np.concatenate(outs, axis=0)


def kernel(x, edge_index, W1, b1, W2, b2, W3, b3):
    global _prep, _prog
    x = np.asarray(x, dtype=np.float32)
    if _prep is None:
        _prep = _preprocess(np.asarray(edge_index))
        K, R, gidx, S = _prep
        _prog = _build_program(K, R)
    K, R, gidx, S = _prep

    h = _run_layer(_prog, x, gidx, S, np.asarray(W1, np.float32))
    h = np.maximum(h + np.asarray(b1, np.float32), 0.0)
    h = _run_layer(_prog, h, gidx, S, np.asarray(W2, np.float32))
    h = np.maximum(h + np.asarray(b2, np.float32), 0.0)
    h = _run_layer(_prog, h, gidx, S, np.asarray(W3, np.float32))
    h = h + np.asarray(b3, np.float32)
    return h.astype(np.float32)



# revision 4
# speedup vs baseline: 8.5529x; 8.5529x over previous
import numpy as np
import ml_dtypes

# GCN 3-layer Trainium2 kernel — 8 cores, single launch, on-device gather.
#
# Aggregate-first GCN: act_next = relu((A_hat @ act) @ W + b).
# dst-node rows sharded 8 ways (12500/core, padded to 12544).
# Per core, per layer:
#   * dma_gather (gpsimd SWDGE 'mlp' firmware) fetches per-edge source rows
#     from a replicated bf16 activation table in DRAM. int16 gather indices
#     force 4 source-row ranges of 25088 rows.
#   * DVE scales gathered rows by the per-edge GCN norm (pads carry norm=0),
#     builds one-hot scatter blocks S from dst-local ids vs an iota ramp.
#   * TensorE: psum[k,n] += xg_chunk^T @ S_chunk over a window's chunks
#     (scatter-add), then aggT @ W dense transform per 128-row window.
#   * DVE adds bias (+relu for layers 1-2); shard written to DRAM; AllGather
#     collective replicates activations for the next layer.
# Edge chunks are padded to the max count over all 8 cores per (window,range)
# so one SPMD program serves every core (only the data differs).

N = 100000
F = 128
NC = 8
NPC = N // NC
WIN = 128
NW = (NPC + WIN - 1) // WIN
NPAD = NW * WIN
NFULL = NC * NPAD
NR = 4
BW = 7                      # windows per gather block
MAXG = 8192                 # max gather indices per SWDGE instruction (carveout)
REPLICATE_IDX = True

_cache = {}
EXEC_NS = []


def _range_size():
    return (NFULL + NR - 1) // NR


def _preprocess(edge_index):
    RANGE = _range_size()
    NB = NW // BW
    src = np.asarray(edge_index[0], dtype=np.int64)
    dst = np.asarray(edge_index[1], dtype=np.int64)
    loop = np.arange(N, dtype=np.int64)
    src = np.concatenate([src, loop])
    dst = np.concatenate([dst, loop])
    deg = np.bincount(dst, minlength=N).astype(np.float32)
    dinv = np.where(deg > 0, 1.0 / np.sqrt(deg), 0.0).astype(np.float32)
    norm = (dinv[src] * dinv[dst]).astype(np.float32)

    srcp = (src // NPC) * NPAD + (src % NPC)      # padded table row
    core = dst // NPC
    dl = dst - core * NPC
    w = dl // WIN
    b = w // BW
    r = srcp // RANGE
    srcl = (srcp % RANGE).astype(np.int64)
    dwin = (dl % WIN).astype(np.float32)

    order = np.lexsort((w, r, b, core))
    core_s, w_s, r_s = core[order], w[order], r[order]
    srcl_s, dwin_s, norm_s = srcl[order], dwin[order], norm[order]

    gid = (core_s * NW + w_s) * NR + r_s
    cnt = np.bincount(gid, minlength=NC * NW * NR).reshape(NC, NW, NR)
    kwr = (cnt.max(axis=0) + 127) // 128          # [NW, NR]
    KMAX = int(kwr.sum(axis=1).max())
    NCH = int(kwr.sum())
    R_total = NCH * 128

    group_off = np.zeros((NW, NR), dtype=np.int64)
    off = 0
    per_block_off = []
    per_block_chunks = []
    gather_list = []
    for bb in range(NB):
        per_block_off.append(off)
        blk = 0
        glist = []
        for rr in range(NR):
            first = off + blk
            n_idx = 0
            for ww in range(bb * BW, (bb + 1) * BW):
                group_off[ww, rr] = off + blk
                blk += int(kwr[ww, rr])
                n_idx += int(kwr[ww, rr]) * 128
            # split into <=MAXG-index SWDGE instructions
            cur = first
            rem = n_idx
            while rem > 0:
                take = min(rem, MAXG)
                glist.append((rr, cur, take))
                cur += take // 128
                rem -= take
        gather_list.append(glist)
        per_block_chunks.append(blk)
        off += blk
    assert off == NCH

    win_segs = [
        [(int(group_off[ww, rr]), int(kwr[ww, rr]))
         for rr in range(NR) if kwr[ww, rr] > 0]
        for ww in range(NW)
    ]

    # rank of each edge within its (core,window,range) group; groups are
    # contiguous in the sorted order but not in gid order, so derive starts
    # from run boundaries of the sorted gid sequence.
    change = np.r_[True, gid[1:] != gid[:-1]]
    group_first = np.flatnonzero(change)
    sizes = np.diff(np.r_[group_first, len(gid)])
    rank = np.arange(len(gid)) - np.repeat(group_first, sizes)
    slot = group_off[w_s, r_s] * 128 + rank

    idx_all = np.zeros((NC, R_total), dtype=np.int16)
    dstl_all = np.zeros((NC, R_total), dtype=np.float32)
    norm_all = np.zeros((NC, R_total), dtype=np.float32)
    idx_all[core_s, slot] = srcl_s.astype(np.int16)
    dstl_all[core_s, slot] = dwin_s
    norm_all[core_s, slot] = norm_s

    bf = ml_dtypes.bfloat16
    idxw = np.zeros((NC, 128, R_total // 16), dtype=np.int16)
    wrapped = idx_all.reshape(NC, R_total // 16, 16).transpose(0, 2, 1)
    for k in range(8 if REPLICATE_IDX else 1):
        idxw[:, k * 16:(k + 1) * 16, :] = wrapped
    dstl_sb = np.ascontiguousarray(
        dstl_all.reshape(NC, NCH, 128).transpose(0, 2, 1)).astype(bf)
    norm_sb = np.ascontiguousarray(
        norm_all.reshape(NC, NCH, 128).transpose(0, 2, 1)).astype(bf)

    meta = dict(KMAX=KMAX, NCH=NCH, R_total=R_total, NB=NB,
                per_block_chunks=per_block_chunks,
                per_block_off=per_block_off,
                gather_list=gather_list, win_segs=win_segs)
    return meta, idxw, dstl_sb, norm_sb


def _build_program(meta):
    import concourse.bass as bass
    import concourse.mybir as mybir
    from concourse import bacc
    from concourse.tile import TileContext

    RANGE = _range_size()
    KMAX = meta["KMAX"]
    NCH = meta["NCH"]
    R_total = meta["R_total"]
    NB = meta["NB"]
    per_block_chunks = meta["per_block_chunks"]
    per_block_off = meta["per_block_off"]
    gather_list = meta["gather_list"]
    win_segs = meta["win_segs"]
    CBMAX = max(per_block_chunks)
    ICOLS = R_total // 16

    nc = bacc.Bacc(None, target_bir_lowering=False, num_devices=NC)
    bf16 = mybir.dt.bfloat16
    i16 = mybir.dt.int16
    f32 = mybir.dt.float32

    xsh_d = nc.dram_tensor("xsh", [NPAD, F], bf16, kind="ExternalInput")
    idx_d = nc.dram_tensor("idx", [128, ICOLS], i16, kind="ExternalInput")
    dstl_d = nc.dram_tensor("dstl", [128, NCH], bf16, kind="ExternalInput")
    norm_d = nc.dram_tensor("normv", [128, NCH], bf16, kind="ExternalInput")
    W_d = nc.dram_tensor("W", [128, 3 * F], bf16, kind="ExternalInput")
    bias_d = nc.dram_tensor("bias", [128, 3 * F], f32, kind="ExternalInput")
    iota_d = nc.dram_tensor("iota", [128, KMAX * F], bf16, kind="ExternalInput")
    out_d = nc.dram_tensor("out", [NPAD, F], f32, kind="ExternalOutput")
    act_a = nc.dram_tensor("act_a", [NFULL, F], bf16)
    act_b = nc.dram_tensor("act_b", [NFULL, F], bf16)
    shard = nc.dram_tensor("shard", [NPAD, F], bf16)

    rg = [list(range(NC))]

    with TileContext(nc) as tc:
        with (
            tc.tile_pool(name="res", bufs=1) as res,
            tc.tile_pool(name="xgp", bufs=2) as xgp,
            tc.tile_pool(name="swp", bufs=2) as swp,
            tc.tile_pool(name="smal", bufs=3) as smal,
            tc.tile_pool(name="psp", bufs=2, space="PSUM") as psp,
        ):
            gat_reg = nc.gpsimd.alloc_register()
            idx_s = res.tile([128, ICOLS], i16)
            dstl_s = res.tile([128, NCH], bf16)
            norm_s = res.tile([128, NCH], bf16)
            W_s = res.tile([128, 3 * F], bf16)
            bias_s = res.tile([128, 3 * F], f32)
            iota_s = res.tile([128, KMAX * F], bf16)
            nc.sync.dma_start(out=idx_s[:, :], in_=idx_d[:, :])
            nc.sync.dma_start(out=dstl_s[:, :], in_=dstl_d[:, :])
            nc.sync.dma_start(out=norm_s[:, :], in_=norm_d[:, :])
            nc.sync.dma_start(out=W_s[:, :], in_=W_d[:, :])
            nc.sync.dma_start(out=bias_s[:, :], in_=bias_d[:, :])
            nc.sync.dma_start(out=iota_s[:, :], in_=iota_d[:, :])

            nc.sync.dma_start(out=shard[:, :], in_=xsh_d[:, :])
            nc.gpsimd.collective_compute(
                "AllGather", mybir.AluOpType.bypass, replica_groups=rg,
                ins=[shard.ap().opt()], outs=[act_a.ap().opt()],
            )

            for l in range(3):
                src_t = act_a if l % 2 == 0 else act_b
                for b in range(NB):
                    off0 = per_block_off[b]
                    cb = per_block_chunks[b]
                    xg_t = xgp.tile([128, CBMAX * F], bf16, tag="xg")
                    for (rr, ch0, n_idx) in gather_list[b]:
                        c0 = ch0 - off0
                        nc.gpsimd.reg_mov(gat_reg, n_idx)
                        nc.gpsimd.dma_gather(
                            out_ap=xg_t[:, c0 * F:(c0 + n_idx // 128) * F]
                            .rearrange("p (c f) -> p c f", f=F),
                            in_ap=src_t[rr * RANGE:(rr + 1) * RANGE, :],
                            idxs_ap=idx_s[:, ch0 * 8:ch0 * 8 + n_idx // 16],
                            num_idxs=n_idx,
                            num_idxs_reg=gat_reg,
                            elem_size=F,
                            single_packet=False,
                        )
                    nc.vector.tensor_tensor(
                        out=xg_t[:, :cb * F],
                        in0=xg_t[:, :cb * F],
                        in1=norm_s[:, off0:off0 + cb].to_broadcast([128, cb, F]),
                        op=mybir.AluOpType.mult,
                    )
                    for wi in range(BW):
                        w = b * BW + wi
                        segs = win_segs[w]
                        kw = sum(k for _, k in segs)
                        S_t = swp.tile([128, KMAX * F], bf16, tag="S")
                        pos = 0
                        for (chpos, k) in segs:
                            nc.vector.tensor_tensor(
                                out=S_t[:, pos * F:(pos + k) * F],
                                in0=dstl_s[:, chpos:chpos + k]
                                .to_broadcast([128, k, F]),
                                in1=iota_s[:, :k * F],
                                op=mybir.AluOpType.is_equal,
                            )
                            pos += k
                        ps1 = psp.tile([128, F], f32, tag="ps1")
                        done = 0
                        pos = 0
                        for (chpos, k) in segs:
                            for c in range(k):
                                nc.tensor.matmul(
                                    ps1[:, :],
                                    xg_t[:, (chpos - off0 + c) * F:
                                         (chpos - off0 + c + 1) * F],
                                    S_t[:, (pos + c) * F:(pos + c + 1) * F],
                                    start=(done == 0),
                                    stop=(done == kw - 1),
                                )
                                done += 1
                            pos += k
                        aggT_t = smal.tile([128, F], bf16, tag="aggT")
                        nc.vector.tensor_copy(out=aggT_t[:, :], in_=ps1[:, :])
                        ps2 = psp.tile([128, F], f32, tag="ps2")
                        nc.tensor.matmul(
                            ps2[:, :], aggT_t[:, :],
                            W_s[:, l * F:(l + 1) * F],
                            start=True, stop=True,
                        )
                        if l < 2:
                            at = smal.tile([128, F], bf16, tag="at")
                            nc.vector.tensor_add(
                                out=at[:, :], in0=ps2[:, :],
                                in1=bias_s[:, l * F:(l + 1) * F])
                            nc.vector.tensor_scalar_max(at[:, :], at[:, :], 0.0)
                            nc.sync.dma_start(
                                out=shard[w * WIN:(w + 1) * WIN, :],
                                in_=at[:, :])
                        else:
                            ot = smal.tile([128, F], f32, tag="ot")
                            nc.vector.tensor_add(
                                out=ot[:, :], in0=ps2[:, :],
                                in1=bias_s[:, l * F:(l + 1) * F])
                            nc.sync.dma_start(
                                out=out_d[w * WIN:(w + 1) * WIN, :],
                                in_=ot[:, :])
                if l < 2:
                    dst_t = act_b if l % 2 == 0 else act_a
                    nc.gpsimd.collective_compute(
                        "AllGather", mybir.AluOpType.bypass, replica_groups=rg,
                        ins=[shard.ap().opt()], outs=[dst_t.ap().opt()],
                    )
    nc.compile()
    return nc


def _prepare(edge_index):
    key = "prep"
    if key not in _cache:
        meta, idxw, dstl_sb, norm_sb = _preprocess(edge_index)
        prog = _build_program(meta)
        _cache[key] = (meta, idxw, dstl_sb, norm_sb, prog)
    return _cache[key]


def kernel(x, edge_index, W1, b1, W2, b2, W3, b3):
    from concourse.bass_utils import run_bass_kernel_spmd

    bf = ml_dtypes.bfloat16
    meta, idxw, dstl_sb, norm_sb, prog = _prepare(edge_index)
    KMAX = meta["KMAX"]

    x = np.asarray(x, dtype=np.float32)
    xpad = np.zeros((NC, NPAD, F), dtype=bf)
    xpad[:, :NPC, :] = x.reshape(NC, NPC, F).astype(bf)

    Wall = np.stack([np.asarray(Wl, np.float32) for Wl in (W1, W2, W3)], 0)
    Wtile = np.concatenate([Wl.astype(bf) for Wl in Wall], axis=1)  # [128, 384]
    ball = [np.asarray(bl, np.float32) for bl in (b1, b2, b3)]
    btile = np.concatenate(
        [np.broadcast_to(bl[None, :], (128, F)) for bl in ball], axis=1
    ).astype(np.float32)
    iota = np.tile(np.arange(F, dtype=np.float32), KMAX)[None, :]
    iota = np.broadcast_to(iota, (128, KMAX * F)).astype(bf)

    in_maps = []
    for c in range(NC):
        in_maps.append({
            "xsh": np.ascontiguousarray(xpad[c]),
            "idx": np.ascontiguousarray(idxw[c]),
            "dstl": np.ascontiguousarray(dstl_sb[c]),
            "normv": np.ascontiguousarray(norm_sb[c]),
            "W": np.ascontiguousarray(Wtile),
            "bias": np.ascontiguousarray(btile),
            "iota": np.ascontiguousarray(iota),
        })
    import time
    t0 = time.perf_counter_ns()
    res = run_bass_kernel_spmd(prog, in_maps, list(range(NC)))
    t1 = time.perf_counter_ns()
    EXEC_NS.append(res.exec_time_ns if getattr(res, "exec_time_ns", None)
                   else t1 - t0)
    outs = []
    for c in range(NC):
        r = res.results[c]
        if isinstance(r, dict):
            r = r["out"]
        elif isinstance(r, (list, tuple)):
            r = r[0]
        outs.append(np.asarray(r)[:NPC])
    return np.concatenate(outs, axis=0).astype(np.float32)
